# revision 1
# baseline (speedup 1.0000x reference)
"""Trainium2 Bass kernel for nn_CustomCLIP_11407433138213 (moe_routing).

Math (per sample b with domain n = labels[b]):
    h   = relu(x @ W1[n])                 [R]
    a   = relu(h @ W2[n])                 [D]
    f   = 0.2*a + 0.8*x                   [D]
    out = exp(ls) * (f/||f||) @ T^T       [N_TXT]

Device strategy (data-parallel over batch, 8 cores x 2048 rows):
  Everything is computed transposed (samples on the free dim):
    - XT [D, rows] streamed per row-block of 512.
    - mm1: hT[n] = W1[n]^T XT  (PSUM, K-chunks of 128), per expert.
    - g[n] = relu(hT[n]) * bcast(mask[n])   (one DVE scalar_tensor_tensor)
    - mm2: pa[d] = sum_n W2'[n]^T g[n]  with W2' = 0.25*W2 (all experts
      accumulate into one PSUM tile; one-hot masks make it a select).
    - f'[d] = relu(pa[d]) + XT[d]  == (0.2*a + 0.8*x)/0.8 per column.
    - s = colsum(f'^2) via ones-matmul; inv = 1/sqrt(s*exp(-2 ls));
      bcast(inv) via rank-1 matmul; fs = f' * bcast(inv).
    - mm3: logitsT[t] = TT^T fs  -> DRAM [N_TXT_PAD, rows].
  Host: transpose/shard inputs, one-hot masks, 0.25*W2, pad text to 1408,
  then gather logits[rows, txt] from per-core logitsT.
Matmuls run as float32r (full PE rate at N=512; fp32 would be 4x slower).
Emission is software-pipelined: block b+1's mm1/mm2 are emitted between
block b's norm reduction and its use, so the PE never waits on the
ACT/DVE norm chain.
"""

import contextlib
import os
import sys

sys.path.insert(0, "/opt/trn_rl_repo")

import numpy as np

import concourse.bass as bass  # noqa: F401  (registers engine types)
import concourse.mybir as mybir
import concourse.tile as tile
from concourse import bacc
from concourse.bass_utils import run_bass_kernel_spmd

# Problem constants (hardcoded per task contract).
B, D, R, ND, NT = 16384, 1024, 256, 3, 1380
NC = 8                    # cores
BPC = B // NC             # rows per core = 2048
RB = 512                  # row-block (matmul moving dim)
NB = BPC // RB            # row-blocks per core = 4
KD = D // 128             # 8 contraction chunks over D
KR = R // 128             # 2 chunks over R
MR = R // 128             # 2 M-chunks over R
NTP = 1408                # text padded to 11*128
TTI = NTP // 128          # 11 text chunks

F32 = mybir.dt.float32
MM_DT = mybir.dt.float32 if os.environ.get("KMM_DT") == "f32" else mybir.dt.float32r


def _f32(ap):
    """View a matmul-dtype AP as plain fp32 for ACT/DVE consumption."""
    return ap.bitcast(F32) if MM_DT != F32 else ap


def build_program():
    nc = bacc.Bacc(
        "TRN2",
        target_bir_lowering=False,
        debug=False,
        enable_asserts=True,
        num_devices=NC,
    )
    xt = nc.declare_dram_parameter("xt", [D, BPC], MM_DT, isOutput=False)
    msk = nc.declare_dram_parameter("msk", [ND, BPC], F32, isOutput=False)
    w1 = nc.declare_dram_parameter("w1", [ND, D, R], MM_DT, isOutput=False)
    w2 = nc.declare_dram_parameter("w2", [ND, R, D], MM_DT, isOutput=False)
    tt = nc.declare_dram_parameter("tt", [D, NTP], MM_DT, isOutput=False)
    sc = nc.declare_dram_parameter("sc", [1, 1], F32, isOutput=False)
    oc = nc.declare_dram_parameter("oc", [128, 1], MM_DT, isOutput=False)
    orow = nc.declare_dram_parameter("orow", [1, 128], MM_DT, isOutput=False)
    ot = nc.declare_dram_parameter("ot", [NTP, BPC], F32, isOutput=True)

    with tile.TileContext(nc) as tc, contextlib.ExitStack() as ctx:
        cst = ctx.enter_context(tc.tile_pool(name="cst", bufs=1))
        p_xb = ctx.enter_context(tc.tile_pool(name="p_xb", bufs=16))
        p_mb = ctx.enter_context(tc.tile_pool(name="p_mb", bufs=5))
        p_g = ctx.enter_context(tc.tile_pool(name="p_g", bufs=6))
        p_fp = ctx.enter_context(tc.tile_pool(name="p_fp", bufs=16))
        p_sq = ctx.enter_context(tc.tile_pool(name="p_sq", bufs=3))
        p_acc = ctx.enter_context(tc.tile_pool(name="p_acc", bufs=2))
        p_pbs = ctx.enter_context(tc.tile_pool(name="p_pbs", bufs=2))
        p_ob = ctx.enter_context(tc.tile_pool(name="p_ob", bufs=2))
        p_nrm = ctx.enter_context(tc.tile_pool(name="p_nrm", bufs=1))

        ps_h = ctx.enter_context(tc.tile_pool(name="ps_h", bufs=2, space="PSUM"))
        ps_a = ctx.enter_context(tc.tile_pool(name="ps_a", bufs=2, space="PSUM"))
        ps_s = ctx.enter_context(tc.tile_pool(name="ps_s", bufs=1, space="PSUM"))
        ps_l = ctx.enter_context(tc.tile_pool(name="ps_l", bufs=2, space="PSUM"))

        # ---- constant tiles (loads emitted in stages below) -------------
        w1t = [
            [
                cst.tile([128, R], MM_DT, name=f"w1_{n}_{k}", tag=f"w1_{n}_{k}")
                for k in range(KD)
            ]
            for n in range(ND)
        ]
        w2t = [
            [
                cst.tile([128, D], MM_DT, name=f"w2_{n}_{r}", tag=f"w2_{n}_{r}")
                for r in range(KR)
            ]
            for n in range(ND)
        ]
        ttt = [
            cst.tile([128, NTP], MM_DT, name=f"tt_{k}", tag=f"tt_{k}")
            for k in range(KD)
        ]
        ones_col = cst.tile([128, 1], MM_DT, name="ones_col", tag="ones_col")
        ones_row = cst.tile([1, 128], MM_DT, name="ones_row", tag="ones_row")
        sct = cst.tile([1, 1], F32, name="sct", tag="sct")

        # per-block live tiles
        S = [dict() for _ in range(NB)]

        def emit_w1_loads():
            nc.sync.dma_start(ones_col[:], oc[:])
            nc.sync.dma_start(ones_row[:], orow[:])
            nc.sync.dma_start(sct[:], sc[:])

        def emit_w1_n(n):
            for k in range(KD):
                nc.sync.dma_start(w1t[n][k][:], w1[n, k * 128 : (k + 1) * 128, :])

        def emit_w2_loads():
            for n in range(ND):
                for r in range(KR):
                    nc.sync.dma_start(w2t[n][r][:], w2[n, r * 128 : (r + 1) * 128, :])

        def emit_tt_loads():
            for k in range(KD):
                nc.sync.dma_start(ttt[k][:], tt[k * 128 : (k + 1) * 128, :])

        def emit_loads(b):
            c0 = b * RB
            xb = []
            for k in range(KD):
                t = p_xb.tile([128, RB], MM_DT, name="xb", tag="xb")
                nc.sync.dma_start(t[:], xt[k * 128 : (k + 1) * 128, c0 : c0 + RB])
                xb.append(t)
            mb = []
            for n in range(ND):
                t = p_mb.tile([128, RB], F32, name="mb", tag="mb")
                nc.sync.dma_start(
                    t[:],
                    msk[n, c0 : c0 + RB]
                    .rearrange("(a n) -> a n", a=1)
                    .to_broadcast((128, RB)),
                )
                mb.append(t)
            S[b]["xb"] = xb
            S[b]["mb"] = mb

        def emit_mm1_g(b):
            xb, mb = S[b]["xb"], S[b]["mb"]
            g = [[None] * MR for _ in range(ND)]
            for n in range(ND):
                for m in range(MR):
                    ph = ps_h.tile([128, RB], F32, name="ph", tag="ph")
                    for k in range(KD):
                        nc.tensor.matmul(
                            ph[:],
                            w1t[n][k][:, m * 128 : (m + 1) * 128],
                            xb[k][:],
                            start=(k == 0),
                            stop=(k == KD - 1),
                        )
                    gt = p_g.tile([128, RB], MM_DT, name="g", tag="g")
                    nc.vector.scalar_tensor_tensor(
                        gt[:],
                        ph[:],
                        0.0,
                        mb[n][:],
                        mybir.AluOpType.max,
                        mybir.AluOpType.mult,
                    )
                    g[n][m] = gt
            S[b]["g"] = g

        def emit_mm2(b):
            xb, g = S[b]["xb"], S[b]["g"]
            fp = []
            sq = []
            for d in range(KD):
                pa = ps_a.tile([128, RB], F32, name="pa", tag="pa")
                first = True
                for n in range(ND):
                    for r in range(KR):
                        nc.tensor.matmul(
                            pa[:],
                            w2t[n][r][:, d * 128 : (d + 1) * 128],
                            g[n][r][:],
                            start=first,
                            stop=(n == ND - 1 and r == KR - 1),
                        )
                        first = False
                ft = p_fp.tile([128, RB], MM_DT, name="fp", tag="fp")
                nc.vector.scalar_tensor_tensor(
                    ft[:],
                    pa[:],
                    0.0,
                    _f32(xb[d][:]),
                    mybir.AluOpType.max,
                    mybir.AluOpType.add,
                )
                fp.append(ft)
                st = p_sq.tile([128, RB], F32, name="sq", tag="sq")
                nc.scalar.square(st[:], _f32(ft[:]))
                if d == 0:
                    acc = p_acc.tile([128, RB], F32, name="acc", tag="acc")
                    nc.gpsimd.tensor_copy(acc[:], st[:])
                elif d < KD - 1:
                    nc.gpsimd.tensor_add(acc[:], acc[:], st[:])
                else:
                    accm = p_acc.tile([128, RB], MM_DT, name="accm", tag="accm")
                    nc.gpsimd.tensor_add(accm[:], acc[:], st[:])
            S[b]["fp"] = fp
            S[b]["accm"] = accm

        def emit_ps_norm(b):
            accm = S[b]["accm"]
            ps = ps_s.tile([1, RB], F32, name="ps", tag="ps")
            nc.tensor.matmul(ps[:], ones_col[:], accm[:], start=True, stop=True)
            iv = p_nrm.tile([1, RB], MM_DT, name="iv", tag="iv")
            nc.scalar.activation(
                iv[:],
                ps[:],
                mybir.ActivationFunctionType.Abs_reciprocal_sqrt,
                scale=sct[:],
            )
            S[b]["iv"] = iv

        def emit_pb(b):
            iv = S[b]["iv"]
            pb = ps_l.tile([128, RB], F32, name="pl", tag="pl")
            nc.tensor.matmul(pb[:], ones_row[:], iv[:], start=True, stop=True)
            pbs = p_pbs.tile([128, RB], F32, name="pbs", tag="pbs")
            nc.scalar.copy(pbs[:], pb[:])
            S[b]["pbs"] = pbs

        def emit_mm3(b):
            c0 = b * RB
            fp = S[b]["fp"]
            pbs = S[b]["pbs"]
            for t_i in range(TTI):
                pl = ps_l.tile([128, RB], F32, name="pl", tag="pl")
                for k in range(KD):
                    nc.tensor.matmul(
                        pl[:],
                        ttt[k][:, t_i * 128 : (t_i + 1) * 128],
                        fp[k][:],
                        start=(k == 0),
                        stop=(k == KD - 1),
                    )
                ob = p_ob.tile([128, RB], F32, name="ob", tag="ob")
                nc.vector.tensor_mul(ob[:], pl[:], pbs[:])
                nc.sync.dma_start(
                    ot[t_i * 128 : (t_i + 1) * 128, c0 : c0 + RB], ob[:]
                )
            # drop per-block state (frees python refs only)
            S[b].clear()

        # ---- emission schedule (software pipelined) ---------------------
        # First wave: interleave W1[0] with xb(0) so the PE can start as
        # soon as the first weight + x chunks land.
        emit_w1_loads()
        emit_w1_n(0)
        emit_loads(0)
        emit_w1_n(1)
        emit_w1_n(2)
        emit_w2_loads()
        emit_mm1_g(0)
        emit_tt_loads()
        emit_mm2(0)
        emit_ps_norm(0)
        for b in range(NB):
            if b + 1 < NB:
                emit_loads(b + 1)
                emit_mm1_g(b + 1)
            emit_pb(b)
            if b + 1 < NB:
                emit_mm2(b + 1)
            emit_mm3(b)
            if b + 1 < NB:
                emit_ps_norm(b + 1)

    nc.compile()
    return nc


_NC_CACHE = None


def _get_program():
    global _NC_CACHE
    if _NC_CACHE is None:
        _NC_CACHE = build_program()
    return _NC_CACHE


def make_in_maps(image_features, domain_labels, W1, W2, text_features, logit_scale):
    image_features = np.asarray(image_features, dtype=np.float32)
    labels = np.asarray(domain_labels)
    W1 = np.asarray(W1, dtype=np.float32)
    W2 = np.asarray(W2, dtype=np.float32)
    text_features = np.asarray(text_features, dtype=np.float32)
    ls = float(np.asarray(logit_scale))

    # Host-side shard prep.
    xt_full = np.ascontiguousarray(image_features.T)            # [D, B]
    masks = np.zeros((ND, B), dtype=np.float32)
    masks[labels.astype(np.int64), np.arange(B)] = 1.0
    w2s = (0.25 * W2).astype(np.float32)                        # fold 0.2/0.8
    tt_full = np.zeros((D, NTP), dtype=np.float32)
    tt_full[:, :NT] = text_features.T
    sc = np.array([[np.exp(-2.0 * ls)]], dtype=np.float32)
    oc = np.ones((128, 1), dtype=np.float32)
    orow = np.ones((1, 128), dtype=np.float32)

    in_maps = []
    for c in range(NC):
        cols = slice(c * BPC, (c + 1) * BPC)
        in_maps.append(
            {
                "xt": np.ascontiguousarray(xt_full[:, cols]),
                "msk": np.ascontiguousarray(masks[:, cols]),
                "w1": W1,
                "w2": w2s,
                "tt": tt_full,
                "sc": sc,
                "oc": oc,
                "orow": orow,
            }
        )
    return in_maps


def kernel(image_features, domain_labels, W1, W2, text_features, logit_scale, **kw):
    in_maps = make_in_maps(
        image_features, domain_labels, W1, W2, text_features, logit_scale
    )
    nc = _get_program()
    res = run_bass_kernel_spmd(nc, in_maps, list(range(NC)))

    out = np.empty((B, NT), dtype=np.float32)
    for c in range(NC):
        out[c * BPC : (c + 1) * BPC, :] = res.results[c]["ot"][:NT, :].T
    return out

